# revision 11
# baseline (speedup 1.0000x reference)
"""LocalVariation kernel for Trainium2 (8 NeuronCores, data-parallel over batch).

out[b, k, y, x] = x[b, 0, y, x] - xp[b, 0, y + di, x + dj]   (replicate pad)
for the 24 off-center (di, dj) offsets of a 5x5 window.

Sharding: batch 16 -> 2 images per core. The host pre-pads each image to
[516, 516] (replicate). The device program is built to minimize instruction
count (the execution environment is dominated by fixed per-instruction cost):

  - ONE 5.2-MiB DMA per image loads T[p, c, i, x] = xpad[128c + p + i, x]
    (the overlapping-window source AP merges (i, x) into one contiguous dim).
  - ONE DVE tensor_sub per 128-row chunk computes all 25 (i, j) blocks at
    once via a 3-free-dim window access pattern (the center block is zeros
    and is simply not stored).
  - TWO 3-MiB stores per chunk (12 channels each), alternating between the
    sync and scalar HWDGE rings.
"""

import numpy as np

import concourse.bass as bass
import concourse.bacc as bacc
import concourse.mybir as mybir
import concourse.tile as tile
from concourse.bass_utils import run_bass_kernel_spmd

N_CORES = 8
B_FULL = 16
BPC = B_FULL // N_CORES  # images per core
H = W = 512
KSZ = 5
PAD = 2
NBR = KSZ * KSZ - 1  # 24
HP = H + 2 * PAD  # 516
WP = W + 2 * PAD  # 516
F32 = mybir.dt.float32
NCH = H // 128  # 4 chunks per image
CBLK = KSZ * WP  # free elems per (chunk) block in T: 2580

_NC_CACHE = {}


def _build_image(nc, tin, tout, x, out, b):
    # One load for the whole image: T[p, c, i*WP + x] = xpad[b, 128c + p + i, x]
    T = tin.tile([128, NCH, CBLK], F32, name=f"T_{b}", tag="T")
    pstep = T.ap[0][0]
    nc.gpsimd.dma_start(
        out=T[:, :, :],
        in_=bass.AP(
            x, b * HP * WP, [[WP, 128], [128 * WP, NCH], [WP, KSZ], [1, WP]]
        ),
    )

    for c in range(NCH):
        # O[p, 5i+j, x] = center - T[p, c, i, j + x]  (one DVE op, FD=12800)
        O = tout.tile([128, KSZ * KSZ, W], F32, name=f"O_{b}_{c}", tag="O")
        ostep = O.ap[0][0]
        tbase = T.offset + c * CBLK
        center = bass.AP(
            T.tensor, tbase + PAD * WP + PAD, [[pstep, 128], [0, KSZ], [0, KSZ], [1, W]]
        )
        win = bass.AP(T.tensor, tbase, [[pstep, 128], [WP, KSZ], [1, KSZ], [1, W]])
        o3 = bass.AP(O.tensor, O.offset, [[ostep, 128], [KSZ * W, KSZ], [W, KSZ], [1, W]])
        nc.vector.tensor_sub(o3, center, win)

        # two stores (channels 0..11 from blocks 0..11, 12..23 from 13..24)
        ooff = b * NBR * H * W + 128 * c * W
        eng1, eng2 = (nc.sync, nc.scalar) if c % 2 == 0 else (nc.scalar, nc.sync)
        eng1.dma_start(
            out=bass.AP(out, ooff, [[W, 128], [H * W, 12], [1, W]]),
            in_=O[:, 0:12, :],
        )
        eng2.dma_start(
            out=bass.AP(out, ooff + 12 * H * W, [[W, 128], [H * W, 12], [1, W]]),
            in_=O[:, 13:25, :],
        )


def build(reps=1, tiny_out=False):
    """tiny_out=True: bench variant — full-size stores go to an Internal DRAM
    tensor (same HBM traffic) and only a [128, 512] probe is an ExternalOutput,
    so per-call transfer over the axon tunnel is negligible."""
    nc = bacc.Bacc("TRN2", target_bir_lowering=False, debug=False, num_devices=N_CORES)
    x = nc.dram_tensor("x", [BPC, HP, WP], F32, kind="ExternalInput")
    out_kind = "Internal" if tiny_out else "ExternalOutput"
    out = nc.dram_tensor("out", [BPC, NBR, H, W], F32, kind=out_kind)
    probe = (
        nc.dram_tensor("probe", [128, W], F32, kind="ExternalOutput") if tiny_out else None
    )
    with tile.TileContext(nc) as tc:
        with (
            tc.tile_pool(name="tin", bufs=1) as tin,
            tc.tile_pool(name="tout", bufs=3) as tout,
        ):
            for _ in range(reps):
                for b in range(BPC):
                    _build_image(nc, tin, tout, x, out, b)
            if probe is not None:
                pt = tin.tile([128, W], F32, name="pt", tag="pt")
                nc.sync.dma_start(out=pt[:, :], in_=bass.AP(out, 0, [[W, 128], [1, W]]))
                nc.sync.dma_start(out=probe.ap(), in_=pt[:, :])
    nc.compile()
    return nc


def _get_nc():
    if "nc" not in _NC_CACHE:
        _NC_CACHE["nc"] = build()
    return _NC_CACHE["nc"]


def pad_input(x):
    """[16, 1, 512, 512] -> replicate-padded [16, 516, 516], float32."""
    xs = np.asarray(x, dtype=np.float32).reshape(B_FULL, H, W)
    return np.pad(xs, ((0, 0), (PAD, PAD), (PAD, PAD)), mode="edge")


def run(x, trace=False):
    nc = _get_nc()
    xp = pad_input(x)
    in_maps = [
        {"x": np.ascontiguousarray(xp[BPC * i : BPC * (i + 1)])} for i in range(N_CORES)
    ]
    res = run_bass_kernel_spmd(nc, in_maps, core_ids=list(range(N_CORES)), trace=trace)
    full = np.concatenate([res.results[i]["out"] for i in range(N_CORES)], axis=0)
    return full.reshape(B_FULL, NBR, H, W), res


def kernel(x):
    return run(x)[0]
